# revision 7
# baseline (speedup 1.0000x reference)
"""Distributed cross-attention kernel for 8 TRN2 NeuronCores (v2).

Problem: q from target, k/v from reference; B=2, N=M=2048, C=1024,
H=16 heads, hd=64; output projection with bias.

Sharding: core c owns heads {2c, 2c+1} for both batches (head
parallel); after attention, one AllToAll per batch redistributes
x^T so core c owns output rows [c*256, (c+1)*256) with all 1024
channels; core c applies the replicated Wproj + bias.

v2 design (vs v1 baseline at ~320us):
- m-tile = 256 queries = exactly one AllToAll slot (clean staging).
- Scores for one kc-pair of BOTH heads live in one [128, 1024] PSUM
  tile (2 banks): T0 (rows 0-63, head0) and T8 (64-127, head1) run
  their K=64 QK^T matmuls concurrently into different banks; ONE
  [128,1024] exp per pair keeps ACT (the bottleneck engine, ~147us
  of exp) running back-to-back with only 2 score tiles ping-ponged.
- AV accumulates h0 and h1 into column halves of a single PSUM bank
  ([65, 512] po tile, ones-column gives softmax denominators): legal
  because the h1 accumulation group is deferred by one m-tile (groups
  in a bank must be sequential), which also smooths PE load.
- Normalization without DRAM roundtrips: po -> pb (SBUF), DVE
  reciprocal of the denominator row, PE ones-matmul broadcasts 1/d
  across partitions, one DVE multiply, and the staging DMA reshapes
  [64, 2x256] -> [128, 256] on the way to the AllToAll buffer.
- Batch-1 QKV and batch-0 proj are emitted as small granules between
  attention pairs so PE slack under the exp stream absorbs them; two
  spare PSUM banks ("w") host those accumulations.
- Startup: xref/xtgt stream in 512-column quarters so K/V blocks
  start after ~1MB of DMA; first exp at ~15-25us instead of ~75us.
"""

import functools

import numpy as np

B = 2
N = 2048  # reference rows (keys)
M = 2048  # target rows (queries)
C = 1024
H = 16
HD = 64
NCORES = 8
HPC = 2  # heads per core
CHPC = HPC * HD  # 128 channels per core
MT = 256  # m-tile = queries per a2a slot
NMT = M // MT  # 8 m-tiles per batch
KC = N // 128  # 16 key chunks
CC = C // 128  # 8 contraction chunks
NPAIR = KC // 2  # 8 kc-pairs per m-tile


@functools.lru_cache(maxsize=1)
def _build():
    import concourse.bacc as bacc
    import concourse.mybir as mybir
    import concourse.tile as tile

    fp32 = mybir.dt.float32
    bf16 = mybir.dt.bfloat16
    AF = mybir.ActivationFunctionType

    nc = bacc.Bacc("TRN2", target_bir_lowering=False, debug=False, num_devices=NCORES)

    xrefT = nc.dram_tensor("xrefT", [B, C, N], bf16, kind="ExternalInput")
    xtgtT = nc.dram_tensor("xtgtT", [B, C, M], bf16, kind="ExternalInput")
    wq = nc.dram_tensor("wq", [C, CHPC], bf16, kind="ExternalInput")
    wk = nc.dram_tensor("wk", [C, CHPC], bf16, kind="ExternalInput")
    wv = nc.dram_tensor("wv", [C, CHPC], bf16, kind="ExternalInput")
    wproj = nc.dram_tensor("wproj", [C, C], bf16, kind="ExternalInput")
    bproj = nc.dram_tensor("bproj", [C], fp32, kind="ExternalInput")
    out = nc.dram_tensor("out", [B, C, MT], fp32, kind="ExternalOutput")

    scale = float(HD) ** -0.5

    with tile.TileContext(nc) as tc:
        with (
            tc.tile_pool(name="wpool", bufs=1) as wpool,
            tc.tile_pool(name="xpool", bufs=24) as xpool,
            tc.tile_pool(name="kqv", bufs=1) as kqv,
            tc.tile_pool(name="epool", bufs=10) as epool,
            tc.tile_pool(name="spool", bufs=2) as spool,
            tc.tile_pool(name="ppool", bufs=2) as ppool,
            tc.tile_pool(name="psS", bufs=2, space="PSUM") as psS,
            tc.tile_pool(name="psP", bufs=2, space="PSUM") as psP,
            tc.tile_pool(name="psW", bufs=2, space="PSUM") as psW,
            tc.tile_pool(name="dram", bufs=1, space="DRAM") as dpool,
        ):
            # ---------------- static tiles ----------------
            wq_sb = wpool.tile([128, CC, CHPC], bf16)
            wk_sb = wpool.tile([128, CC, CHPC], bf16)
            wv_sb = wpool.tile([128, CC, CHPC], bf16)
            wp_sb = wpool.tile([128, CC, C], bf16)
            bias_f = wpool.tile([1, C], fp32)
            biasb = wpool.tile([1, C], bf16)
            ones = wpool.tile([128, MT], bf16)

            kT = [kqv.tile([128, N], bf16, tag=f"kT{b}", name=f"kT{b}") for b in range(B)]
            qT = [kqv.tile([128, M], bf16, tag=f"qT{b}", name=f"qT{b}") for b in range(B)]
            vA = [
                kqv.tile([128, KC, HPC, HD + 1], bf16, tag=f"vA{b}", name=f"vA{b}")
                for b in range(B)
            ]

            a2a_in = [
                dpool.tile([NCORES, CHPC, MT], bf16, tag=f"a2a_in{b}", name=f"a2a_in{b}")
                for b in range(B)
            ]
            a2a_out = [
                dpool.tile([NCORES, CHPC, MT], bf16, tag=f"a2a_out{b}", name=f"a2a_out{b}")
                for b in range(B)
            ]
            bar_in = dpool.tile([NCORES, 1], fp32, tag="bar_in", name="bar_in")
            bar_out = dpool.tile([NCORES, 1], fp32, tag="bar_out", name="bar_out")

            # ---------------- phase 0: loads ----------------
            # launch-skew barrier while only DMAs run
            nc.gpsimd.collective_compute(
                "AllToAll",
                mybir.AluOpType.bypass,
                replica_groups=[list(range(NCORES))],
                ins=[bar_in[:].opt()],
                outs=[bar_out[:].opt()],
            )
            for cc in range(CC):
                nc.gpsimd.dma_start(wk_sb[:, cc, :], wk[cc * 128:(cc + 1) * 128, :])
                nc.gpsimd.dma_start(wv_sb[:, cc, :], wv[cc * 128:(cc + 1) * 128, :])
                nc.gpsimd.dma_start(wq_sb[:, cc, :], wq[cc * 128:(cc + 1) * 128, :])
            nc.vector.memset(ones[:], 1.0)
            for b in range(B):
                nc.vector.memset(vA[b][:, :, :, HD:HD + 1], 1.0)

            xch = {}

            def x_tile(which, b, cc):
                key = (which, b, cc)
                if key not in xch:
                    xch[key] = xpool.tile(
                        [128, N], bf16, tag="x", name=f"x{which}{b}_{cc}"
                    )
                return xch[key]

            # batch-0: quarter-column loads so compute starts after ~1MB
            def load_qtr(which, b, cc, qtr):
                src = xrefT if which == "r" else xtgtT
                t = x_tile(which, b, cc)
                cols = slice(qtr * 512, (qtr + 1) * 512)
                eng = nc.sync if cc % 2 == 0 else nc.gpsimd
                eng.dma_start(t[:, cols], src[b, cc * 128:(cc + 1) * 128, cols])

            for qtr in range(4):
                for cc in range(CC):
                    load_qtr("r", 0, cc, qtr)
            for qtr in range(4):
                for cc in range(CC):
                    load_qtr("t", 0, cc, qtr)
            # batch-1 xref: full-chunk loads into fresh slots (DMA runs
            # ahead during batch-0). batch-1 xtgt REUSES batch-0 xref
            # slots, so those loads are emitted only after the startup
            # QKV blocks that read batch-0 data (see below).
            for cc in range(CC):
                t = x_tile("r", 1, cc)
                eng = nc.sync if cc % 2 == 0 else nc.gpsimd
                eng.dma_start(t[:], xrefT[1, cc * 128:(cc + 1) * 128, :])
            # wproj + bias (needed mid batch-1)
            for cc in range(CC):
                nc.gpsimd.dma_start(wp_sb[:, cc, :], wproj[cc * 128:(cc + 1) * 128, :])
            nc.sync.dma_start(bias_f[:], bproj.ap().rearrange("(a c) -> a c", a=1))
            nc.vector.tensor_copy(biasb[:], bias_f[:])

            # ---------------- QKV building blocks ----------------
            # Each block accumulates in a [128, 512] "w" PSUM tile and
            # evicts via DVE. Emitted either inline (startup) or as a
            # list of granule closures (interleaved with attention).

            def kt_block_granules(b, w_sb, dstT, blk, which):
                """4 granules of 2 matmuls; dstT[:, blk*512:(blk+1)*512]."""
                state = {}

                def g(i):
                    def run():
                        if i == 0:
                            state["W"] = psW.tile(
                                [128, 512], fp32, tag="w", name=f"w{which}{b}{blk}"
                            )
                        W = state["W"]
                        for cc in (2 * i, 2 * i + 1):
                            nc.tensor.matmul(
                                W[:],
                                lhsT=w_sb[:, cc, :],
                                rhs=xch[(which, b, cc)][:, blk * 512:(blk + 1) * 512],
                                start=(cc == 0),
                                stop=(cc == CC - 1),
                            )
                        if i == 3:
                            nc.vector.tensor_copy(
                                dstT[:, blk * 512:(blk + 1) * 512], W[:]
                            )
                    return run

                return [g(i) for i in range(4)]

            def v_block_granules(b, blk):
                """4 granules; fills vA[b] chunks blk*4 .. blk*4+3."""
                state = {}

                def g(i):
                    def run():
                        if i == 0:
                            state["W"] = psW.tile(
                                [128, 512], fp32, tag="w", name=f"wv{b}{blk}"
                            )
                        W = state["W"]
                        # granule i: chunk j=i, all 8 cc
                        j = i
                        kc = blk * 4 + j
                        for cc in range(CC):
                            nc.tensor.matmul(
                                W[:, j * 128:(j + 1) * 128],
                                lhsT=xch[("r", b, cc)][:, kc * 128:(kc + 1) * 128],
                                rhs=wv_sb[:, cc, :],
                                start=(cc == 0),
                                stop=(cc == CC - 1),
                            )
                        if i == 3:
                            nc.vector.tensor_copy(
                                vA[b][:, blk * 4:blk * 4 + 4, :, 0:HD],
                                W[:].rearrange("p (k h d) -> p k h d", k=4, h=HPC),
                            )
                    return run

                return [g(i) for i in range(4)]

            def run_block(granules):
                for g in granules:
                    g()

            # ---------------- proj granules ----------------
            y_sb = {}

            def proj_granules(b):
                """Granule list: y load, then per-oc matmul/bias/evict."""
                gs = []

                def y_load():
                    y_sb[b] = ppool.tile(
                        [128, NCORES, MT], bf16, tag="y", name=f"y{b}"
                    )
                    for i in range(NCORES):
                        nc.sync.dma_start(y_sb[b][:, i, :], a2a_out[b][i])

                gs.append(y_load)
                state = {}

                def mk(oc, part):
                    def run():
                        if part == 0:
                            state[oc] = psW.tile(
                                [128, 512], fp32, tag="w", name=f"wp{b}{oc}"
                            )
                        W = state[oc]
                        ps = W[:, 0:MT]
                        rng = range(4) if part == 0 else range(4, CC)
                        for cci in rng:
                            nc.tensor.matmul(
                                ps,
                                lhsT=wp_sb[:, cci, oc * 128:(oc + 1) * 128],
                                rhs=y_sb[b][:, cci, :],
                                start=(cci == 0),
                                stop=False,
                            )
                        if part == 1:
                            nc.tensor.matmul(
                                ps,
                                lhsT=biasb[0:1, oc * 128:(oc + 1) * 128],
                                rhs=ones[0:1, :],
                                start=False,
                                stop=True,
                            )
                            osb = spool.tile([128, MT], fp32, tag="osb", name="osb")
                            nc.vector.tensor_copy(osb[:], ps)
                            nc.sync.dma_start(out[b, oc * 128:(oc + 1) * 128, :], osb[:])
                    return run

                for oc in range(CC):
                    gs.append(mk(oc, 0))
                    gs.append(mk(oc, 1))
                return gs

            # ---------------- attention ----------------
            # Pair state: eS tiles indexed by (mt, p); po tiles by mt.
            eS_t = {}
            po_t = {}

            def qk_exp(b, mt, p):
                kc = 2 * p
                S = psS.tile([128, 1024], fp32, tag="S", name="S")
                for h in range(HPC):
                    for j in range(2):
                        nc.tensor.matmul(
                            S[:, (2 * h + j) * MT:(2 * h + j + 1) * MT],
                            lhsT=kT[b][h * HD:(h + 1) * HD, (kc + j) * 128:(kc + j + 1) * 128],
                            rhs=qT[b][h * HD:(h + 1) * HD, mt * MT:(mt + 1) * MT],
                            start=True,
                            stop=True,
                        )
                eS = epool.tile([128, 4, MT], bf16, tag="eS", name="eS")
                nc.scalar.activation(
                    eS[:].rearrange("p a b -> p (a b)"), S[:], AF.Exp, scale=scale
                )
                eS_t[(b, mt, p)] = eS

            def av(b, mt, p, h):
                """AV for head h of (mt, p) into po_t[(b, mt)] col half."""
                po = po_t[(b, mt)]
                eS = eS_t[(b, mt, p)]
                kc = 2 * p
                for j in range(2):
                    nc.tensor.matmul(
                        po[:, h * MT:(h + 1) * MT],
                        lhsT=vA[b][:, kc + j, h, :],
                        rhs=eS[:, 2 * h + j, :],
                        start=(p == 0 and j == 0),
                        stop=(p == NPAIR - 1 and j == 1),
                    )

            def norm_stage(b, mt):
                """Evict po, normalize, stage into a2a_in[b][mt]."""
                po = po_t.pop((b, mt))
                pb = spool.tile([HD + 1, 512], fp32, tag="pb", name="pb")
                nc.vector.tensor_copy(pb[:], po[:])
                rb = spool.tile([HD + 1, 512], bf16, tag="rb", name="rb")
                with nc.allow_low_precision(reason="1/denominator to bf16 (~4e-3)"):
                    nc.vector.reciprocal(rb[HD:HD + 1, :], pb[HD:HD + 1, :])
                rbc = psW.tile([128, 512], fp32, tag="w", name="rbc")
                for h in range(HPC):
                    nc.tensor.matmul(
                        rbc[0:HD, h * MT:(h + 1) * MT],
                        lhsT=ones[HD:HD + 1, 0:HD],
                        rhs=rb[HD:HD + 1, h * MT:(h + 1) * MT],
                        start=True,
                        stop=True,
                    )
                xst = spool.tile([HD, 512], bf16, tag="xst", name="xst")
                nc.vector.tensor_mul(xst[:], pb[0:HD, :], rbc[0:HD, :])
                # [64, 2*256] -> [128, 256]: DMA reshapes across partitions
                nc.sync.dma_start(a2a_in[b][mt][0:HD, :], xst[:, 0:MT])
                nc.sync.dma_start(a2a_in[b][mt][HD:CHPC, :], xst[:, MT:2 * MT])

            def fire_a2a(b):
                nc.gpsimd.collective_compute(
                    "AllToAll",
                    mybir.AluOpType.bypass,
                    replica_groups=[list(range(NCORES))],
                    ins=[a2a_in[b][:].opt()],
                    outs=[a2a_out[b][:].opt()],
                )

            # ---------------- emission schedule ----------------
            # startup: batch-0 K and V per quarter, then q block 0
            for blk in range(4):
                run_block(kt_block_granules(0, wk_sb, kT[0], blk, "r"))
                run_block(v_block_granules(0, blk))
            run_block(kt_block_granules(0, wq_sb, qT[0], 0, "t"))

            # batch-1 xtgt loads: slots reuse batch-0 xref tiles, whose
            # readers (the startup blocks above) are now all emitted.
            for cc in range(CC):
                t = x_tile("t", 1, cc)
                eng = nc.sync if cc % 2 == 0 else nc.gpsimd
                eng.dma_start(t[:], xtgtT[1, cc * 128:(cc + 1) * 128, :])

            # granule queue consumed one per attention pair
            gq = []
            for blk in range(1, 4):
                gq.extend(kt_block_granules(0, wq_sb, qT[0], blk, "t"))
            for blk in range(4):
                gq.extend(kt_block_granules(1, wk_sb, kT[1], blk, "r"))
                gq.extend(v_block_granules(1, blk))
            for blk in range(4):
                gq.extend(kt_block_granules(1, wq_sb, qT[1], blk, "t"))
            # batch-1 granule stream: proj(0) after a2a(0) has finished
            gq1 = [None] * (3 * NPAIR)
            gq1.extend(proj_granules(0))

            GPAIR = B * NMT * NPAIR  # 128 pairs total

            for g in range(GPAIR):
                b = g // (NMT * NPAIR)
                mt = (g // NPAIR) % NMT
                p = g % NPAIR
                if p == 0:
                    po_t[(b, mt)] = psP.tile([HD + 1, 512], fp32, tag="po", name="po")
                qk_exp(b, mt, p)
                # deferred AV work: h0 of previous pair, h1 of previous m-tile
                if p > 0:
                    av(b, mt, p - 1, 0)
                else:
                    pb_, pmt = (b, mt - 1) if mt > 0 else (b - 1, NMT - 1)
                    if pmt >= 0 and (pb_, pmt, NPAIR - 1) in eS_t:
                        av(pb_, pmt, NPAIR - 1, 0)
                pb_, pmt = (b, mt - 1) if mt > 0 else (b - 1, NMT - 1)
                if pmt >= 0 and (pb_, pmt) in po_t:
                    av(pb_, pmt, p, 1)
                    if p == NPAIR - 1:
                        norm_stage(pb_, pmt)
                        if pmt == NMT - 1:
                            fire_a2a(pb_)
                # interleaved granule
                queue = gq if b == 0 else gq1
                idx = g % (NMT * NPAIR)
                if idx < len(queue) and queue[idx] is not None:
                    queue[idx]()

            # flush: last m-tile's deferred work
            b, mt = B - 1, NMT - 1
            av(b, mt, NPAIR - 1, 0)
            for p in range(NPAIR):
                av(b, mt, p, 1)
            norm_stage(b, mt)
            fire_a2a(b)
            for gr in proj_granules(1):
                gr()

    nc.compile()
    return nc


def _shard_inputs(reference_data, target_data, Wq, Wkv, Wproj, bproj):
    import ml_dtypes

    bf16 = ml_dtypes.bfloat16
    xrefT = np.ascontiguousarray(
        np.asarray(reference_data, dtype=np.float32).transpose(0, 2, 1)
    ).astype(bf16)
    xtgtT = np.ascontiguousarray(
        np.asarray(target_data, dtype=np.float32).transpose(0, 2, 1)
    ).astype(bf16)
    Wq = np.asarray(Wq, dtype=np.float32)
    Wkv = np.asarray(Wkv, dtype=np.float32)
    Wproj_b = np.asarray(Wproj, dtype=np.float32).astype(bf16)
    bproj = np.asarray(bproj, dtype=np.float32)

    in_maps = []
    for c in range(NCORES):
        lo, hi = c * CHPC, (c + 1) * CHPC
        in_maps.append(
            {
                "xrefT": xrefT,
                "xtgtT": xtgtT,
                "wq": Wq[:, lo:hi].astype(bf16),
                "wk": Wkv[:, lo:hi].astype(bf16),
                "wv": Wkv[:, C + lo:C + hi].astype(bf16),
                "wproj": Wproj_b,
                "bproj": bproj,
            }
        )
    return in_maps


def _ensure_ntff_hook():
    """Register the axon NTFF profile hook if the image's antenv lacks it."""
    try:
        import antenv.axon_hooks  # noqa: F401

        return
    except ImportError:
        pass
    import sys
    import types

    import antenv

    mod = types.ModuleType("antenv.axon_hooks")
    state = {"hook": None}
    mod.set_axon_ntff_profile_hook = lambda h: state.__setitem__("hook", h)
    mod.get_axon_ntff_profile_hook = lambda: state["hook"]
    sys.modules["antenv.axon_hooks"] = mod
    antenv.axon_hooks = mod
    try:
        from trn_agent_boot.trn_boot import _ntff_profile_via_ctypes

        mod.set_axon_ntff_profile_hook(
            _ntff_profile_via_ctypes("/opt/axon/libaxon_pjrt.so")
        )
    except Exception:
        pass


def run(inputs: dict, trace: bool = False):
    """Compile (cached), run on 8 cores, return (full_output, results)."""
    from concourse.bass_utils import run_bass_kernel_spmd

    if trace:
        _ensure_ntff_hook()
    nc = _build()
    in_maps = _shard_inputs(**inputs)
    res = run_bass_kernel_spmd(
        nc, in_maps, core_ids=list(range(NCORES)), trace=trace
    )
    return _assemble(res), res


def _assemble(res):
    full = np.zeros((B, M, C), dtype=np.float32)
    for c in range(NCORES):
        blk = np.asarray(res.results[c]["out"], dtype=np.float32)  # [B, C, MT]
        for b in range(B):
            full[b, c * MT:(c + 1) * MT, :] = blk[b].T
    return full


def kernel(reference_data, target_data, Wq, Wkv, Wproj, bproj) -> np.ndarray:
    full, _ = run(
        {
            "reference_data": reference_data,
            "target_data": target_data,
            "Wq": Wq,
            "Wkv": Wkv,
            "Wproj": Wproj,
            "bproj": bproj,
        }
    )
    return full


# revision 14
# speedup vs baseline: 1.0692x; 1.0692x over previous
"""Distributed cross-attention kernel for 8 TRN2 NeuronCores (v2).

Problem: q from target, k/v from reference; B=2, N=M=2048, C=1024,
H=16 heads, hd=64; output projection with bias.

Sharding: core c owns heads {2c, 2c+1} for both batches (head
parallel); after attention, one AllToAll per batch redistributes
x^T so core c owns output rows [c*256, (c+1)*256) with all 1024
channels; core c applies the replicated Wproj + bias.

v2 design (vs v1 baseline at ~320us):
- m-tile = 256 queries = exactly one AllToAll slot (clean staging).
- Scores for one kc-pair of BOTH heads live in one [128, 1024] PSUM
  tile (2 banks): T0 (rows 0-63, head0) and T8 (64-127, head1) run
  their K=64 QK^T matmuls concurrently into different banks; ONE
  [128,1024] exp per pair keeps ACT (the bottleneck engine, ~147us
  of exp) running back-to-back with only 2 score tiles ping-ponged.
- AV accumulates h0 and h1 into column halves of a single PSUM bank
  ([65, 512] po tile, ones-column gives softmax denominators): legal
  because the h1 accumulation group is deferred by one m-tile (groups
  in a bank must be sequential), which also smooths PE load.
- Normalization without DRAM roundtrips: po -> pb (SBUF), DVE
  reciprocal of the denominator row, PE ones-matmul broadcasts 1/d
  across partitions, one DVE multiply, and the staging DMA reshapes
  [64, 2x256] -> [128, 256] on the way to the AllToAll buffer.
- Batch-1 QKV and batch-0 proj are emitted as small granules between
  attention pairs so PE slack under the exp stream absorbs them; two
  spare PSUM banks ("w") host those accumulations.
- Startup: xref/xtgt stream in 512-column quarters so K/V blocks
  start after ~1MB of DMA; first exp at ~15-25us instead of ~75us.
"""

import functools

import numpy as np

B = 2
N = 2048  # reference rows (keys)
M = 2048  # target rows (queries)
C = 1024
H = 16
HD = 64
NCORES = 8
HPC = 2  # heads per core
CHPC = HPC * HD  # 128 channels per core
MT = 256  # m-tile = queries per a2a slot
NMT = M // MT  # 8 m-tiles per batch
KC = N // 128  # 16 key chunks
CC = C // 128  # 8 contraction chunks
NPAIR = KC // 2  # 8 kc-pairs per m-tile


@functools.lru_cache(maxsize=1)
def _build():
    import concourse.bacc as bacc
    import concourse.mybir as mybir
    import concourse.tile as tile

    fp32 = mybir.dt.float32
    bf16 = mybir.dt.bfloat16
    AF = mybir.ActivationFunctionType

    nc = bacc.Bacc("TRN2", target_bir_lowering=False, debug=False, num_devices=NCORES)

    xrefT = nc.dram_tensor("xrefT", [B, C, N], bf16, kind="ExternalInput")
    xtgtT = nc.dram_tensor("xtgtT", [B, C, M], bf16, kind="ExternalInput")
    wq = nc.dram_tensor("wq", [C, CHPC], bf16, kind="ExternalInput")
    wk = nc.dram_tensor("wk", [C, CHPC], bf16, kind="ExternalInput")
    wv = nc.dram_tensor("wv", [C, CHPC], bf16, kind="ExternalInput")
    wproj = nc.dram_tensor("wproj", [C, C], bf16, kind="ExternalInput")
    bproj = nc.dram_tensor("bproj", [C], fp32, kind="ExternalInput")
    out = nc.dram_tensor("out", [B, C, MT], fp32, kind="ExternalOutput")

    scale = float(HD) ** -0.5

    with tile.TileContext(nc) as tc:
        with (
            tc.tile_pool(name="wpool", bufs=1) as wpool,
            tc.tile_pool(name="xpool", bufs=24) as xpool,
            tc.tile_pool(name="kqv", bufs=1) as kqv,
            tc.tile_pool(name="epool", bufs=10) as epool,
            tc.tile_pool(name="spool", bufs=2) as spool,
            tc.tile_pool(name="ppool", bufs=2) as ppool,
            tc.tile_pool(name="psS", bufs=2, space="PSUM") as psS,
            tc.tile_pool(name="psP", bufs=2, space="PSUM") as psP,
            tc.tile_pool(name="psW", bufs=2, space="PSUM") as psW,
            tc.tile_pool(name="dram", bufs=1, space="DRAM") as dpool,
        ):
            # ---------------- static tiles ----------------
            wq_sb = wpool.tile([128, CC, CHPC], bf16)
            wk_sb = wpool.tile([128, CC, CHPC], bf16)
            wv_sb = wpool.tile([128, CC, CHPC], bf16)
            wp_sb = wpool.tile([128, CC, C], bf16)
            bias_f = wpool.tile([1, C], fp32)
            biasb = wpool.tile([1, C], bf16)
            ones = wpool.tile([128, MT], bf16)

            kT = [kqv.tile([128, N], bf16, tag=f"kT{b}", name=f"kT{b}") for b in range(B)]
            qT = [kqv.tile([128, M], bf16, tag=f"qT{b}", name=f"qT{b}") for b in range(B)]
            vA = [
                kqv.tile([128, KC, HPC, HD + 1], bf16, tag=f"vA{b}", name=f"vA{b}")
                for b in range(B)
            ]

            a2a_in = [
                dpool.tile([NCORES, CHPC, MT], bf16, tag=f"a2a_in{b}", name=f"a2a_in{b}")
                for b in range(B)
            ]
            a2a_out = [
                dpool.tile([NCORES, CHPC, MT], bf16, tag=f"a2a_out{b}", name=f"a2a_out{b}")
                for b in range(B)
            ]
            bar_in = dpool.tile([NCORES, 1], fp32, tag="bar_in", name="bar_in")
            bar_out = dpool.tile([NCORES, 1], fp32, tag="bar_out", name="bar_out")

            # ---------------- phase 0: loads ----------------
            for cc in range(CC):
                nc.gpsimd.dma_start(wk_sb[:, cc, :], wk[cc * 128:(cc + 1) * 128, :])
                nc.gpsimd.dma_start(wv_sb[:, cc, :], wv[cc * 128:(cc + 1) * 128, :])
                nc.gpsimd.dma_start(wq_sb[:, cc, :], wq[cc * 128:(cc + 1) * 128, :])
            nc.vector.memset(ones[:], 1.0)
            for b in range(B):
                nc.vector.memset(vA[b][:, :, :, HD:HD + 1], 1.0)

            xch = {}

            def x_tile(which, b, cc):
                key = (which, b, cc)
                if key not in xch:
                    xch[key] = xpool.tile(
                        [128, N], bf16, tag="x", name=f"x{which}{b}_{cc}"
                    )
                return xch[key]

            # batch-0: quarter-column loads so compute starts after ~1MB
            def load_qtr(which, b, cc, qtr):
                src = xrefT if which == "r" else xtgtT
                t = x_tile(which, b, cc)
                cols = slice(qtr * 512, (qtr + 1) * 512)
                eng = nc.sync if cc % 2 == 0 else nc.gpsimd
                eng.dma_start(t[:, cols], src[b, cc * 128:(cc + 1) * 128, cols])

            for qtr in range(4):
                for cc in range(CC):
                    load_qtr("r", 0, cc, qtr)
            for qtr in range(4):
                for cc in range(CC):
                    load_qtr("t", 0, cc, qtr)
            # batch-1 xref: full-chunk loads into fresh slots (DMA runs
            # ahead during batch-0). batch-1 xtgt REUSES batch-0 xref
            # slots, so those loads are emitted only after the startup
            # QKV blocks that read batch-0 data (see below).
            for cc in range(CC):
                t = x_tile("r", 1, cc)
                eng = nc.sync if cc % 2 == 0 else nc.gpsimd
                eng.dma_start(t[:], xrefT[1, cc * 128:(cc + 1) * 128, :])
            # wproj + bias (needed mid batch-1)
            for cc in range(CC):
                nc.gpsimd.dma_start(wp_sb[:, cc, :], wproj[cc * 128:(cc + 1) * 128, :])
            nc.sync.dma_start(bias_f[:], bproj.ap().rearrange("(a c) -> a c", a=1))
            nc.vector.tensor_copy(biasb[:], bias_f[:])
            # launch-skew barrier: emitted after the load DMAs so it does
            # not block the gpsimd DMA queue at kernel start
            nc.gpsimd.collective_compute(
                "AllToAll",
                mybir.AluOpType.bypass,
                replica_groups=[list(range(NCORES))],
                ins=[bar_in[:].opt()],
                outs=[bar_out[:].opt()],
            )

            # ---------------- QKV building blocks ----------------
            # Each block accumulates in a [128, 512] "w" PSUM tile and
            # evicts via DVE. Emitted either inline (startup) or as a
            # list of granule closures (interleaved with attention).

            def kt_block_granules(b, w_sb, dstT, blk, which):
                """4 granules of 2 matmuls; dstT[:, blk*512:(blk+1)*512]."""
                state = {}

                def g(i):
                    def run():
                        if i == 0:
                            state["W"] = psW.tile(
                                [128, 512], fp32, tag="w", name=f"w{which}{b}{blk}"
                            )
                        W = state["W"]
                        for cc in (2 * i, 2 * i + 1):
                            nc.tensor.matmul(
                                W[:],
                                lhsT=w_sb[:, cc, :],
                                rhs=xch[(which, b, cc)][:, blk * 512:(blk + 1) * 512],
                                start=(cc == 0),
                                stop=(cc == CC - 1),
                            )
                        if i == 3:
                            nc.vector.tensor_copy(
                                dstT[:, blk * 512:(blk + 1) * 512], W[:]
                            )
                    return run

                return [g(i) for i in range(4)]

            def v_block_granules(b, blk):
                """4 granules; fills vA[b] chunks blk*4 .. blk*4+3."""
                state = {}

                def g(i):
                    def run():
                        if i == 0:
                            state["W"] = psW.tile(
                                [128, 512], fp32, tag="w", name=f"wv{b}{blk}"
                            )
                        W = state["W"]
                        # granule i: chunk j=i, all 8 cc
                        j = i
                        kc = blk * 4 + j
                        for cc in range(CC):
                            nc.tensor.matmul(
                                W[:, j * 128:(j + 1) * 128],
                                lhsT=xch[("r", b, cc)][:, kc * 128:(kc + 1) * 128],
                                rhs=wv_sb[:, cc, :],
                                start=(cc == 0),
                                stop=(cc == CC - 1),
                            )
                        if i == 3:
                            nc.vector.tensor_copy(
                                vA[b][:, blk * 4:blk * 4 + 4, :, 0:HD],
                                W[:].rearrange("p (k h d) -> p k h d", k=4, h=HPC),
                            )
                    return run

                return [g(i) for i in range(4)]

            def run_block(granules):
                for g in granules:
                    g()

            # ---------------- proj granules ----------------
            y_sb = {}

            def proj_granules(b):
                """Granule list: y load, then per-oc matmul/bias/evict."""
                gs = []

                def y_load():
                    y_sb[b] = ppool.tile(
                        [128, NCORES, MT], bf16, tag="y", name=f"y{b}"
                    )
                    for i in range(NCORES):
                        eng = nc.sync if i % 2 == 0 else nc.gpsimd
                        eng.dma_start(y_sb[b][:, i, :], a2a_out[b][i])

                gs.append(y_load)
                state = {}

                def mk(oc, part):
                    def run():
                        if part == 0:
                            state[oc] = psW.tile(
                                [128, 512], fp32, tag="w", name=f"wp{b}{oc}"
                            )
                        W = state[oc]
                        ps = W[:, 0:MT]
                        rng = range(4) if part == 0 else range(4, CC)
                        for cci in rng:
                            nc.tensor.matmul(
                                ps,
                                lhsT=wp_sb[:, cci, oc * 128:(oc + 1) * 128],
                                rhs=y_sb[b][:, cci, :],
                                start=(cci == 0),
                                stop=False,
                            )
                        if part == 1:
                            nc.tensor.matmul(
                                ps,
                                lhsT=biasb[0:1, oc * 128:(oc + 1) * 128],
                                rhs=ones[0:1, :],
                                start=False,
                                stop=True,
                            )
                            osb = spool.tile([128, MT], fp32, tag="osb", name="osb")
                            nc.vector.tensor_copy(osb[:], ps)
                            nc.sync.dma_start(out[b, oc * 128:(oc + 1) * 128, :], osb[:])
                    return run

                for oc in range(CC):
                    gs.append(mk(oc, 0))
                    gs.append(mk(oc, 1))
                return gs

            # ---------------- attention ----------------
            # Pair state: eS tiles indexed by (mt, p); po tiles by mt.
            eS_t = {}
            po_t = {}

            def qk_exp(b, mt, p):
                kc = 2 * p
                S = psS.tile([128, 1024], fp32, tag="S", name="S")
                for h in range(HPC):
                    for j in range(2):
                        nc.tensor.matmul(
                            S[:, (2 * h + j) * MT:(2 * h + j + 1) * MT],
                            lhsT=kT[b][h * HD:(h + 1) * HD, (kc + j) * 128:(kc + j + 1) * 128],
                            rhs=qT[b][h * HD:(h + 1) * HD, mt * MT:(mt + 1) * MT],
                            start=True,
                            stop=True,
                        )
                eS = epool.tile([128, 4, MT], bf16, tag="eS", name="eS")
                nc.scalar.activation(
                    eS[:].rearrange("p a b -> p (a b)"), S[:], AF.Exp, scale=scale
                )
                eS_t[(b, mt, p)] = eS

            def av(b, mt, p, h):
                """AV for head h of (mt, p) into po_t[(b, mt)] col half."""
                po = po_t[(b, mt)]
                eS = eS_t[(b, mt, p)]
                kc = 2 * p
                for j in range(2):
                    nc.tensor.matmul(
                        po[:, h * MT:(h + 1) * MT],
                        lhsT=vA[b][:, kc + j, h, :],
                        rhs=eS[:, 2 * h + j, :],
                        start=(p == 0 and j == 0),
                        stop=(p == NPAIR - 1 and j == 1),
                    )

            norm_t = {}

            def norm_a(b, mt):
                """Evict po and start the reciprocal (slow DVE op). The
                rest of the normalization runs one m-tile later so the
                PE never waits on the reciprocal (a stalled PE idles HAM
                into the half-rate clock state)."""
                po = po_t.pop((b, mt))
                pb = spool.tile([HD + 1, 512], fp32, tag="pb", name="pb")
                nc.vector.tensor_copy(pb[:], po[:])
                rb = spool.tile([HD + 1, 512], bf16, tag="rb", name="rb")
                with nc.allow_low_precision(reason="1/denominator to bf16 (~4e-3)"):
                    nc.vector.reciprocal(rb[HD:HD + 1, :], pb[HD:HD + 1, :])
                norm_t[(b, mt)] = (pb, rb)

            def norm_b(b, mt):
                """Broadcast 1/d across partitions (PE), apply, stage."""
                pb, rb = norm_t.pop((b, mt))
                rbc = psW.tile([128, 512], fp32, tag="w", name="rbc")
                for h in range(HPC):
                    nc.tensor.matmul(
                        rbc[0:HD, h * MT:(h + 1) * MT],
                        lhsT=ones[HD:HD + 1, 0:HD],
                        rhs=rb[HD:HD + 1, h * MT:(h + 1) * MT],
                        start=True,
                        stop=True,
                    )
                xst = spool.tile([HD, 512], bf16, tag="xst", name="xst")
                nc.vector.tensor_mul(xst[:], pb[0:HD, :], rbc[0:HD, :])
                # [64, 2*256] -> [128, 256]: two DMAs reshape across partitions
                nc.sync.dma_start(a2a_in[b][mt][0:HD, :], xst[:, 0:MT])
                nc.sync.dma_start(a2a_in[b][mt][HD:CHPC, :], xst[:, MT:2 * MT])

            def fire_a2a(b):
                nc.gpsimd.collective_compute(
                    "AllToAll",
                    mybir.AluOpType.bypass,
                    replica_groups=[list(range(NCORES))],
                    ins=[a2a_in[b][:].opt()],
                    outs=[a2a_out[b][:].opt()],
                )

            # ---------------- emission schedule ----------------
            # startup: batch-0 K and V per quarter, then q block 0
            for blk in range(4):
                run_block(kt_block_granules(0, wk_sb, kT[0], blk, "r"))
                run_block(v_block_granules(0, blk))
            run_block(kt_block_granules(0, wq_sb, qT[0], 0, "t"))

            # batch-1 xtgt loads: slots reuse batch-0 xref tiles, whose
            # readers (the startup blocks above) are now all emitted.
            for cc in range(CC):
                t = x_tile("t", 1, cc)
                eng = nc.sync if cc % 2 == 0 else nc.gpsimd
                eng.dma_start(t[:], xtgtT[1, cc * 128:(cc + 1) * 128, :])

            # granule queue consumed one per attention pair
            gq = []
            for blk in range(1, 4):
                gq.extend(kt_block_granules(0, wq_sb, qT[0], blk, "t"))
            for blk in range(4):
                gq.extend(kt_block_granules(1, wk_sb, kT[1], blk, "r"))
                gq.extend(v_block_granules(1, blk))
            for blk in range(4):
                gq.extend(kt_block_granules(1, wq_sb, qT[1], blk, "t"))
            # batch-1 granule stream: proj(0) after a2a(0) has finished
            # (a2a(0) fires at the end of b1 m-tile 1; give it ~2 m-tiles)
            gq1 = [None] * (4 * NPAIR)
            gq1.extend(proj_granules(0))

            GPAIR = B * NMT * NPAIR  # 128 pairs total

            for g in range(GPAIR):
                b = g // (NMT * NPAIR)
                mt = (g // NPAIR) % NMT
                p = g % NPAIR
                if p == 0:
                    po_t[(b, mt)] = psP.tile([HD + 1, 512], fp32, tag="po", name="po")
                qk_exp(b, mt, p)
                # deferred AV work: h0 of previous pair, h1 of previous m-tile
                if p > 0:
                    av(b, mt, p - 1, 0)
                else:
                    pb_, pmt = (b, mt - 1) if mt > 0 else (b - 1, NMT - 1)
                    if pmt >= 0 and (pb_, pmt, NPAIR - 1) in eS_t:
                        av(pb_, pmt, NPAIR - 1, 0)
                pb_, pmt = (b, mt - 1) if mt > 0 else (b - 1, NMT - 1)
                if pmt >= 0 and (pb_, pmt) in po_t:
                    av(pb_, pmt, p, 1)
                    if p == NPAIR - 1:
                        norm_a(pb_, pmt)
                if p == NPAIR - 1:
                    gmt = (b * NMT + mt) - 2
                    if gmt >= 0:
                        b2, mt2 = divmod(gmt, NMT)
                        if (b2, mt2) in norm_t:
                            norm_b(b2, mt2)
                            if mt2 == NMT - 1:
                                fire_a2a(b2)
                # interleaved granule
                queue = gq if b == 0 else gq1
                idx = g % (NMT * NPAIR)
                if idx < len(queue) and queue[idx] is not None:
                    queue[idx]()

            # flush: last m-tile's deferred work
            b, mt = B - 1, NMT - 1
            av(b, mt, NPAIR - 1, 0)
            for p in range(NPAIR):
                av(b, mt, p, 1)
            norm_a(b, mt)
            norm_b(b, mt - 1)
            norm_b(b, mt)
            fire_a2a(b)
            for gr in proj_granules(1):
                gr()

    nc.compile()
    return nc


def _shard_inputs(reference_data, target_data, Wq, Wkv, Wproj, bproj):
    import ml_dtypes

    bf16 = ml_dtypes.bfloat16
    xrefT = np.ascontiguousarray(
        np.asarray(reference_data, dtype=np.float32).transpose(0, 2, 1)
    ).astype(bf16)
    xtgtT = np.ascontiguousarray(
        np.asarray(target_data, dtype=np.float32).transpose(0, 2, 1)
    ).astype(bf16)
    Wq = np.asarray(Wq, dtype=np.float32)
    Wkv = np.asarray(Wkv, dtype=np.float32)
    Wproj_b = np.asarray(Wproj, dtype=np.float32).astype(bf16)
    bproj = np.asarray(bproj, dtype=np.float32)

    in_maps = []
    for c in range(NCORES):
        lo, hi = c * CHPC, (c + 1) * CHPC
        in_maps.append(
            {
                "xrefT": xrefT,
                "xtgtT": xtgtT,
                "wq": Wq[:, lo:hi].astype(bf16),
                "wk": Wkv[:, lo:hi].astype(bf16),
                "wv": Wkv[:, C + lo:C + hi].astype(bf16),
                "wproj": Wproj_b,
                "bproj": bproj,
            }
        )
    return in_maps


def _ensure_ntff_hook():
    """Register the axon NTFF profile hook if the image's antenv lacks it."""
    try:
        import antenv.axon_hooks  # noqa: F401

        return
    except ImportError:
        pass
    import sys
    import types

    import antenv

    mod = types.ModuleType("antenv.axon_hooks")
    state = {"hook": None}
    mod.set_axon_ntff_profile_hook = lambda h: state.__setitem__("hook", h)
    mod.get_axon_ntff_profile_hook = lambda: state["hook"]
    sys.modules["antenv.axon_hooks"] = mod
    antenv.axon_hooks = mod
    try:
        from trn_agent_boot.trn_boot import _ntff_profile_via_ctypes

        mod.set_axon_ntff_profile_hook(
            _ntff_profile_via_ctypes("/opt/axon/libaxon_pjrt.so")
        )
    except Exception:
        pass


def run(inputs: dict, trace: bool = False):
    """Compile (cached), run on 8 cores, return (full_output, results)."""
    from concourse.bass_utils import run_bass_kernel_spmd

    if trace:
        _ensure_ntff_hook()
    nc = _build()
    in_maps = _shard_inputs(**inputs)
    res = run_bass_kernel_spmd(
        nc, in_maps, core_ids=list(range(NCORES)), trace=trace
    )
    return _assemble(res), res


def _assemble(res):
    full = np.zeros((B, M, C), dtype=np.float32)
    for c in range(NCORES):
        blk = np.asarray(res.results[c]["out"], dtype=np.float32)  # [B, C, MT]
        for b in range(B):
            full[b, c * MT:(c + 1) * MT, :] = blk[b].T
    return full


def kernel(reference_data, target_data, Wq, Wkv, Wproj, bproj) -> np.ndarray:
    full, _ = run(
        {
            "reference_data": reference_data,
            "target_data": target_data,
            "Wq": Wq,
            "Wkv": Wkv,
            "Wproj": Wproj,
            "bproj": bproj,
        }
    )
    return full
